# revision 12
# baseline (speedup 1.0000x reference)
"""Trainium2 Bass kernel for nn_DSBlock (diff_pool -> DGCNN -> diff_unpool).

Data-parallel over batch B=16 across 8 NeuronCores (2 batch elements per
core).  BatchNorm batch statistics are exchanged with three tiny
AllReduces (input-BN, DGCNN BN1, DGCNN BN2).

Self-contained: hardcodes shapes from the problem spec.
"""
import sys
sys.path.insert(0, '/opt/trn_rl_repo')

import numpy as np
import ml_dtypes

import concourse.bass as bass
import concourse.bacc as bacc
import concourse.tile as tile
from concourse import mybir
from concourse.bass_utils import run_bass_kernel_spmd
from concourse.alu_op_type import AluOpType

B, C, N, K = 16, 128, 8192, 256
NCORES, BLOC = 8, 2
INE, BNE = 1e-3, 1e-5
NT = N // 512          # 16 n-tiles of 512
NCH = N // 128         # 64 n-chunks of 128
f32, bf16 = mybir.dt.float32, mybir.dt.bfloat16
AF = mybir.ActivationFunctionType
ALU = AluOpType
BIG_NEG = -1.0e9

_cache: dict = {}
TRACE = False
TRACE_KW: dict = {}
ITERS = 1
LAST_RESULT = None


def _emit(nc, tc, ctx_cfg, iters=1):
    reuse_h, dub_zero = ctx_cfg

    # ---------------- DRAM I/O ----------------
    xb_d = nc.dram_tensor("xb", [BLOC, C, N], bf16, kind="ExternalInput").ap()
    dpw_d = nc.dram_tensor("dpw_t", [C, K], bf16, kind="ExternalInput").ap()
    duw_d = nc.dram_tensor("duw_t", [C, K], bf16, kind="ExternalInput").ap()
    scA_d = nc.dram_tensor("scA_t", [C, C], bf16, kind="ExternalInput").ap()
    scB_d = nc.dram_tensor("scB_t", [C, C], bf16, kind="ExternalInput").ap()
    w1s_d = nc.dram_tensor("w1s_t", [K, K], bf16, kind="ExternalInput").ap()
    w1b_d = nc.dram_tensor("w1b_t", [K, K], bf16, kind="ExternalInput").ap()
    w2_d = nc.dram_tensor("w2_t", [K, K], bf16, kind="ExternalInput").ap()
    idf_d = nc.dram_tensor("identf", [128, 128], f32, kind="ExternalInput").ap()
    idb_d = nc.dram_tensor("identb", [128, 128], bf16, kind="ExternalInput").ap()
    pp_d = nc.dram_tensor("pp", [128, 13], f32, kind="ExternalInput").ap()
    dub_d = nc.dram_tensor("dub_bc", [128, K], f32, kind="ExternalInput").ap()
    out_d = nc.dram_tensor("out", [BLOC, C, N], f32, kind="ExternalOutput").ap()
    for _it in range(iters):
        _emit_iter(nc, tc, reuse_h, dub_zero, xb_d, dpw_d, duw_d, scA_d, scB_d,
                   w1s_d, w1b_d, w2_d, idf_d, idb_d, pp_d, dub_d, out_d)


def _emit_iter(nc, tc, reuse_h, dub_zero, xb_d, dpw_d, duw_d, scA_d, scB_d,
               w1s_d, w1b_d, w2_d, idf_d, idb_d, pp_d, dub_d, out_d):
    # ---------------- persistent pools ----------------
    import contextlib
    es = contextlib.ExitStack()
    consts = es.enter_context(tc.tile_pool(name="consts", bufs=1))
    bigp = es.enter_context(tc.tile_pool(name="bigp", bufs=2))
    xt_p = es.enter_context(tc.tile_pool(name="xt", bufs=1))
    small = es.enter_context(tc.tile_pool(name="small", bufs=1))
    dram = es.enter_context(tc.tile_pool(name="dram", bufs=1, space="DRAM"))
    s2t_p = es.enter_context(tc.tile_pool(name="s2t", bufs=8))
    s2_p = es.enter_context(tc.tile_pool(name="s2", bufs=16))
    et_p = es.enter_context(tc.tile_pool(name="et", bufs=8))
    outp = es.enter_context(tc.tile_pool(name="outp", bufs=3))

    # constants
    dpw = consts.tile([C, K], bf16); nc.sync.dma_start(dpw[:], dpw_d[:])
    duw = consts.tile([C, K], bf16); nc.sync.dma_start(duw[:], duw_d[:])
    scA = consts.tile([C, C], bf16); nc.sync.dma_start(scA[:], scA_d[:])
    scB = consts.tile([C, C], bf16); nc.sync.dma_start(scB[:], scB_d[:])
    w1s = consts.tile([128, 2, K], bf16)
    nc.sync.dma_start(w1s[:], w1s_d.rearrange("(h c) o -> c h o", h=2))
    w1b = consts.tile([128, 2, K], bf16)
    nc.sync.dma_start(w1b[:], w1b_d.rearrange("(h c) o -> c h o", h=2))
    w2 = consts.tile([128, 2, K], bf16)
    nc.sync.dma_start(w2[:], w2_d.rearrange("(h c) o -> c h o", h=2))
    idf = consts.tile([128, 128], f32); nc.sync.dma_start(idf[:], idf_d[:])
    idb = consts.tile([128, 128], bf16); nc.sync.dma_start(idb[:], idb_d[:])
    pp = consts.tile([128, 13], f32); nc.sync.dma_start(pp[:], pp_d[:])
    if not dub_zero:
        dub = consts.tile([128, K], f32); nc.sync.dma_start(dub[:], dub_d[:])
    ones_bf = consts.tile([128, 1], bf16); nc.vector.memset(ones_bf[:], 1.0)
    ones_f = consts.tile([128, 1], f32); nc.vector.memset(ones_f[:], 1.0)

    DPG, DPB, DUG, DUB_, SCB = 0, 1, 2, 3, 4
    G1_, BB1, G2_, BB2 = 5, 7, 9, 11  # +h for half

    # per-b persistent tiles
    xb = [bigp.tile([C, N], bf16, tag="xb", name=f"xb{b_}") for b_ in range(BLOC)]
    h = [bigp.tile([C, N], bf16, tag="h", name=f"h{b_}") for b_ in range(BLOC)]
    xT = [xt_p.tile([128, NCH, 128], bf16, tag="xT", name=f"xT{b_}") for b_ in range(BLOC)]

    for b in range(BLOC):
        for q in range(4):
            qs = slice(q * 2048, (q + 1) * 2048)
            nc.sync.dma_start(xb[b][:, qs], xb_d[b][:, qs])
        nc.sync.dma_start_transpose(xT[b][:], xb[b][:])

    # ---------------- IN stats + AR1 ----------------
    stats = small.tile([128, BLOC, 16, 6], f32)
    mv = small.tile([128, BLOC, 2], f32)
    tvb = small.tile([128, BLOC], f32)   # v + INE
    prt = small.tile([128, BLOC + 1], f32)
    for b in range(BLOC):
        for t in range(16):
            nc.vector.bn_stats(stats[:, b, t, :], xb[b][:, t * 512:(t + 1) * 512])
        nc.vector.bn_aggr(mv[:, b, :], stats[:, b, :, :])
        nc.vector.tensor_scalar(tvb[:, b:b + 1], mv[:, b, 1:2], INE, None, ALU.add)
        nc.vector.reciprocal(prt[:, b:b + 1], tvb[:, b:b + 1])
        nc.vector.tensor_tensor(prt[:, b:b + 1], prt[:, b:b + 1], mv[:, b, 1:2], ALU.mult)
    nc.vector.tensor_tensor(prt[:, BLOC:BLOC + 1], prt[:, 0:1], prt[:, 1:2], ALU.add)
    ar1_i = dram.tile([128, 1], f32)
    ar1_o = dram.tile([128, 1], f32)
    nc.sync.dma_start(ar1_i[:], prt[:, BLOC:BLOC + 1])
    nc.gpsimd.collective_compute("AllReduce", mybir.AluOpType.add,
                                 replica_groups=[list(range(NCORES))],
                                 ins=[ar1_i.opt()], outs=[ar1_o.opt()])
    arv = small.tile([128, 1], f32)
    nc.sync.dma_start(arv[:], ar1_o[:])

    # BN-in affine per (b, branch)
    vb2 = small.tile([128, 1], f32)   # var_c + BNE
    nc.vector.tensor_scalar(vb2[:], arv[:], 1.0 / B, BNE, ALU.mult, ALU.add)
    Adp = small.tile([128, BLOC], f32); Bdp = small.tile([128, BLOC], f32)
    Adu = small.tile([128, BLOC], f32); Bdu = small.tile([128, BLOC], f32)
    wrk = small.tile([128, 4], f32)
    for b in range(BLOC):
        nc.vector.tensor_tensor(wrk[:, 0:1], vb2[:], tvb[:, b:b + 1], ALU.mult)
        nc.scalar.sqrt(wrk[:, 1:2], wrk[:, 0:1])
        nc.vector.reciprocal(wrk[:, 0:1], wrk[:, 1:2])   # R_b
        nc.vector.tensor_scalar(wrk[:, 2:3], mv[:, b, 0:1], -1.0, None, ALU.mult)  # -m
        nc.vector.tensor_tensor(Adp[:, b:b + 1], wrk[:, 0:1], pp[:, DPG:DPG + 1], ALU.mult)
        nc.vector.scalar_tensor_tensor(Bdp[:, b:b + 1], Adp[:, b:b + 1], wrk[:, 2:3],
                                       pp[:, DPB:DPB + 1], ALU.mult, ALU.add)
        if not reuse_h:
            nc.vector.tensor_tensor(Adu[:, b:b + 1], wrk[:, 0:1], pp[:, DUG:DUG + 1], ALU.mult)
            nc.vector.scalar_tensor_tensor(Bdu[:, b:b + 1], Adu[:, b:b + 1], wrk[:, 2:3],
                                           pp[:, DUB_:DUB_ + 1], ALU.mult, ALU.add)

    # ---------------- h (relu(BN(IN(x)))) ----------------
    for b in range(BLOC):
        for t in range(NT):
            sl = slice(t * 512, (t + 1) * 512)
            nc.scalar.activation(h[b][:, sl], xb[b][:, sl], AF.Relu,
                                 bias=Bdp[:, b:b + 1], scale=Adp[:, b:b + 1])
    if reuse_h:
        h2 = h
    else:
        h2 = [bigp.tile([C, N], bf16, tag="h2", name=f"h2_{b_}") for b_ in range(BLOC)]
        for b in range(BLOC):
            for t in range(NT):
                sl = slice(t * 512, (t + 1) * 512)
                nc.scalar.activation(h2[b][:, sl], xb[b][:, sl], AF.Relu,
                                     bias=Bdu[:, b:b + 1], scale=Adu[:, b:b + 1])

    # ---------------- pool branch ----------------
    # per b: embed^T chunks -> exp -> E^T;  Z[k] = sum_n E^T;  x_down = x @ E^T
    feats_f = []   # [2][128,128] f32 per b (k-half, c)
    feats_b = []   # bf16
    f2x_f = []     # 2*feats f32
    rcp1 = []      # [128,1] f32 per (b, half)
    with (
        tc.tile_pool(name="ph_a_ps", bufs=1, space="PSUM") as aps,
        tc.tile_pool(name="ph_a_sb", bufs=2) as phsb,
    ):
        for b in range(BLOC):
            xd = aps.tile([128, K], f32, tag="xd", bufs=2)
            zp = aps.tile([1, K], f32, tag="z", bufs=1)
            for a in range(NCH):
                ep = aps.tile([128, K], f32, tag="emb", bufs=2)
                nc.tensor.matmul(ep[:], h[b][:, a * 128:(a + 1) * 128], dpw[:])
                et = et_p.tile([128, K], bf16, tag="et")
                nc.scalar.activation(et[:], ep[:], AF.Exp)
                nc.tensor.matmul(xd[:], xT[b][:, a, :], et[:],
                                 start=(a == 0), stop=(a == NCH - 1))
                nc.tensor.matmul(zp[:], ones_bf[:], et[:],
                                 start=(a == 0), stop=(a == NCH - 1))
            # Z -> reciprocal row -> per-half columns
            zrow = phsb.tile([1, K], f32, tag="zrow")
            nc.vector.reciprocal(zrow[:], zp[:])
            xd_s = phsb.tile([128, K], f32, tag="xds")
            nc.vector.tensor_copy(xd_s[:], xd[:])
            ff = []; fb = []; f2 = []; rc = []
            for hh in range(2):
                rp = aps.tile([128, 128], f32, tag="tp", bufs=2)
                nc.tensor.transpose(rp[:, 0:1], zrow[:, hh * 128:(hh + 1) * 128], idf[0:1, 0:1])
                rcol = small.tile([128, 1], f32, tag=f"rcp1_{b}_{hh}")
                nc.vector.tensor_copy(rcol[:], rp[:, 0:1])
                rcol2 = small.tile([128, 1], f32, tag=f"rcp1x2_{b}_{hh}")
                nc.vector.tensor_scalar(rcol2[:], rcol[:], 2.0, None, ALU.mult)
                tp = aps.tile([128, 128], f32, tag="tp", bufs=2)
                nc.tensor.transpose(tp[:], xd_s[:, hh * 128:(hh + 1) * 128], idf[:])
                t_f = small.tile([128, 128], f32, tag=f"ff_{b}_{hh}")
                t_b2 = small.tile([128, 128], bf16, tag=f"fb_{b}_{hh}")
                t_2x = small.tile([128, 128], f32, tag=f"f2_{b}_{hh}")
                nc.vector.tensor_scalar(t_f[:], tp[:], rcol[:], None, ALU.mult)
                nc.vector.tensor_scalar(t_b2[:], tp[:], rcol[:], None, ALU.mult)
                nc.vector.tensor_scalar(t_2x[:], tp[:], rcol2[:], None, ALU.mult)
                ff.append(t_f); fb.append(t_b2); f2.append(t_2x); rc.append(rcol)
            feats_f.append(ff); feats_b.append(fb); f2x_f.append(f2); rcp1.append(rc)

    # ---------------- DGCNN part A: knn + conv1 ----------------
    c1 = [[small.tile([128, 6, 128], bf16, tag=f"c1_{b}_{oh}", name=f"c1_{b}_{oh}") for oh in range(2)]
          for b in range(BLOC)]
    st1 = small.tile([128, 2 * BLOC, 2, 6], f32)
    with (
        tc.tile_pool(name="ph_b_ps", bufs=1, space="PSUM") as bps,
        tc.tile_pool(name="ph_b_sb", bufs=2) as bsb,
    ):
        for b in range(BLOC):
            # sq[p] via ones @ feats^2
            fsq = bsb.tile([128, 2, 128], f32, tag="fsq")
            for hh in range(2):
                nc.scalar.square(fsq[:, hh, :], feats_f[b][hh][:])
            sqp = bps.tile([1, 128], f32, tag="sq", bufs=1)
            for hh in range(2):
                nc.tensor.matmul(sqp[:], ones_f[:], fsq[:, hh, :],
                                 start=(hh == 0), stop=(hh == 1))
            negsq = bsb.tile([1, 128], f32, tag="negsq")
            nc.vector.tensor_scalar(negsq[:], sqp[:], -1.0, None, ALU.mult)
            onesr = bsb.tile([1, 128], f32, tag="onesr")
            nc.vector.memset(onesr[:], 1.0)
            pdp = bps.tile([128, 128], f32, tag="pd", bufs=1)
            for hh in range(2):
                nc.tensor.matmul(pdp[:], f2x_f[b][hh][:], feats_f[b][hh][:],
                                 start=(hh == 0), stop=False)
            nc.tensor.matmul(pdp[:], negsq[:], onesr[:], start=False, stop=False)
            nc.tensor.matmul(pdp[:], onesr[:], negsq[:], start=False, stop=True)
            pd_s = bsb.tile([128, 128], f32, tag="pds")
            nc.vector.tensor_copy(pd_s[:], pdp[:])
            # G^T = f^T W1b^T   [p, o=256]
            gtp = bps.tile([128, K], f32, tag="gt", bufs=1)
            for hh in range(2):
                nc.tensor.matmul(gtp[:], feats_b[b][hh][:], w1b[:, hh, :],
                                 start=(hh == 0), stop=(hh == 1))
            GT = bsb.tile([128, K], bf16, tag="GT")
            nc.vector.tensor_copy(GT[:], gtp[:])
            # C = W1s @ f   [o-half, p] x2
            C_s = bsb.tile([128, 2, 128], f32, tag="Cs")
            for oh in range(2):
                cp = bps.tile([128, 128], f32, tag="Cp", bufs=1)
                for hh in range(2):
                    nc.tensor.matmul(cp[:], w1s[:, hh, oh * 128:(oh + 1) * 128],
                                     feats_b[b][hh][:], start=(hh == 0), stop=(hh == 1))
                nc.vector.tensor_copy(C_s[:, oh, :], cp[:])
            # knn masks + n_term + c1
            rmx = bsb.tile([128, 1], f32, tag="rmx")
            for j in range(6):
                nc.vector.reduce_max(rmx[:], pd_s[:], mybir.AxisListType.X)
                mk = bsb.tile([128, 128], bf16, tag="mk")
                nc.vector.tensor_scalar(mk[:], pd_s[:], rmx[:], None, ALU.is_ge)
                if j < 5:
                    nc.vector.scalar_tensor_tensor(pd_s[:], mk[:], BIG_NEG, pd_s[:],
                                                   ALU.mult, ALU.add)
                mtp = bps.tile([128, 128], bf16, tag="mtp", bufs=2)
                nc.tensor.transpose(mtp[:], mk[:], idb[:])
                mkT = bsb.tile([128, 128], bf16, tag="mkT")
                nc.vector.tensor_copy(mkT[:], mtp[:])
                for oh in range(2):
                    ntp = bps.tile([128, 128], f32, tag="ntp", bufs=2)
                    nc.tensor.matmul(ntp[:], GT[:, oh * 128:(oh + 1) * 128], mkT[:])
                    nc.vector.scalar_tensor_tensor(c1[b][oh][:, j, :], ntp[:], -1.0,
                                                   C_s[:, oh, :], ALU.mult, ALU.add)
            for oh in range(2):
                for ch in range(2):
                    nc.vector.bn_stats(st1[:, 2 * b + oh, ch, :],
                                       c1[b][oh][:, :, :].rearrange("p a b -> p (a b)")[:, ch * 384:(ch + 1) * 384])

    # BN1 partial -> AR2
    mv1 = small.tile([128, 2, 2], f32)
    ar2b = small.tile([128, 4], f32)
    for oh in range(2):
        nc.vector.bn_aggr(mv1[:, oh, :], st1[:].rearrange("p (b o) c s -> p o b c s", o=2)[:, oh])
        nc.vector.tensor_scalar(ar2b[:, oh:oh + 1], mv1[:, oh, 0:1], 1536.0, None, ALU.mult)
        nc.vector.scalar_tensor_tensor(ar2b[:, 2 + oh:3 + oh], mv1[:, oh, 0:1],
                                       mv1[:, oh, 0:1], mv1[:, oh, 1:2], ALU.mult, ALU.add)
        nc.vector.tensor_scalar(ar2b[:, 2 + oh:3 + oh], ar2b[:, 2 + oh:3 + oh],
                                1536.0, None, ALU.mult)
    ar2_i = dram.tile([128, 4], f32)
    ar2_o = dram.tile([128, 4], f32)
    nc.sync.dma_start(ar2_i[:], ar2b[:])
    nc.gpsimd.collective_compute("AllReduce", mybir.AluOpType.add,
                                 replica_groups=[list(range(NCORES))],
                                 ins=[ar2_i.opt()], outs=[ar2_o.opt()])

    # ---------------- unpool machinery ----------------
    s2grp = {}
    s2tgrp = {}
    zk = [small.tile([128, NCH], f32, tag=f"zk_{b}", name=f"zk_{b}") for b in range(BLOC)]
    rcp2 = [small.tile([128, NCH], f32, tag=f"rk_{b}", name=f"rk_{b}") for b in range(BLOC)]

    def emit_exp_group(b, g, e2ps_pool, scr_pool):
        s2t = s2t_p.tile([128, 4, K], bf16, tag="s2t", name=f"s2t_{b}_{g}")
        for q in range(4):
            a = g * 4 + q
            ep = e2ps_pool.tile([128, K], f32, tag="e2", bufs=3, name=f"ep_{b}_{a}")
            nc.tensor.matmul(ep[:], h2[b][:, a * 128:(a + 1) * 128], duw[:])
            if dub_zero:
                nc.scalar.activation(s2t[:, q, :], ep[:], AF.Exp,
                                     accum_out=zk[b][:, a:a + 1])
            else:
                sc = scr_pool.tile([128, K], f32, tag="e2s", name=f"sc_{b}_{a}")
                nc.vector.tensor_tensor(sc[:], ep[:], dub[:], ALU.add)
                nc.scalar.activation(s2t[:, q, :], sc[:], AF.Exp,
                                     accum_out=zk[b][:, a:a + 1])
            nc.vector.reciprocal(rcp2[b][:, a:a + 1], zk[b][:, a:a + 1])
            nc.vector.tensor_scalar(s2t[:, q, :], s2t[:, q, :],
                                    rcp2[b][:, a:a + 1], None, ALU.mult)
        s2tgrp[(b, g)] = s2t

    def emit_transpose_group(b, g):
        s2 = s2_p.tile([128, 8, 128], bf16, tag="s2", name=f"s2_{b}_{g}")
        nc.sync.dma_start_transpose(s2[:], s2tgrp[(b, g)][:])
        s2grp[(b, g)] = s2

    def emit_final(b, t, fpool):
        fps = fpool.tile([128, 512], f32, tag="fps", bufs=3, name=f"fps_{b}_{t}")
        nc.tensor.matmul(fps[:], scA[:], xb[b][:, t * 512:(t + 1) * 512],
                         start=True, stop=False)
        s2 = s2grp[(b, t)]
        s2v = s2[:].rearrange("p (a k) q -> p k a q", k=2)
        for kh in range(2):
            nc.tensor.matmul(fps[:], y2[b * 2 + kh][:],
                             s2v[:, kh, :, :], start=False, stop=(kh == 1))
        ot = outp.tile([128, 512], f32, tag="ot", name=f"ot_{b}_{t}")
        if t % 2 == 0:
            nc.scalar.activation(ot[:], fps[:], AF.Identity, bias=pp[:, SCB:SCB + 1])
        else:
            nc.vector.tensor_scalar(ot[:], fps[:], pp[:, SCB:SCB + 1], None, ALU.add)
        nc.sync.dma_start(out_d[b][:, t * 512:(t + 1) * 512], ot[:])

    PREGROUPS = 8  # b=1 exp groups emitted early to overlap AR3 latency

    with (
        tc.tile_pool(name="ph_c_ps", bufs=1, space="PSUM") as cps,
        tc.tile_pool(name="ph_c_sb", bufs=2) as csb,
    ):
        e2ps, c2ps = cps, cps
        for g in range(NT):
            emit_exp_group(0, g, e2ps, csb)
            emit_transpose_group(0, g)

        # -------- BN1 apply + conv2 + BN2 stats + AR3 --------
        ar2r = small.tile([128, 4], f32)
        nc.sync.dma_start(ar2r[:], ar2_o[:])
        a1 = small.tile([128, 2], f32); b1 = small.tile([128, 2], f32)
        wk2 = small.tile([128, 3], f32)
        CNT1 = float(B * 128 * 6)
        for oh in range(2):
            nc.vector.tensor_scalar(wk2[:, 0:1], ar2r[:, oh:oh + 1], 1.0 / CNT1, None, ALU.mult)  # mean
            nc.vector.tensor_scalar(wk2[:, 1:2], ar2r[:, 2 + oh:3 + oh], 1.0 / CNT1, None, ALU.mult)
            nc.vector.scalar_tensor_tensor(wk2[:, 2:3], wk2[:, 0:1], wk2[:, 0:1],
                                           wk2[:, 1:2], ALU.mult, ALU.subtract)
            nc.vector.tensor_scalar(wk2[:, 2:3], wk2[:, 2:3], -1.0, BNE, ALU.mult, ALU.add)  # var+eps
            nc.scalar.sqrt(wk2[:, 1:2], wk2[:, 2:3])
            nc.vector.reciprocal(wk2[:, 1:2], wk2[:, 1:2])
            nc.vector.tensor_tensor(a1[:, oh:oh + 1], wk2[:, 1:2], pp[:, G1_ + oh:G1_ + oh + 1], ALU.mult)
            nc.vector.tensor_scalar(wk2[:, 0:1], wk2[:, 0:1], -1.0, None, ALU.mult)
            nc.vector.scalar_tensor_tensor(b1[:, oh:oh + 1], a1[:, oh:oh + 1], wk2[:, 0:1],
                                           pp[:, BB1 + oh:BB1 + oh + 1], ALU.mult, ALU.add)
        g1 = [[csb.tile([128, 6, 128], bf16, tag=f"g1_{b}_{oh}", name=f"g1_{b}_{oh}", bufs=1) for oh in range(2)]
              for b in range(BLOC)]
        c2 = [[csb.tile([128, 6, 128], bf16, tag=f"c2_{b}_{oh}", name=f"c2_{b}_{oh}", bufs=1) for oh in range(2)]
              for b in range(BLOC)]
        st2 = small.tile([128, 2 * BLOC, 2, 6], f32)
        for b in range(BLOC):
            for oh in range(2):
                nc.scalar.activation(g1[b][oh][:].rearrange("p a b -> p (a b)"),
                                     c1[b][oh][:].rearrange("p a b -> p (a b)"),
                                     AF.Relu, bias=b1[:, oh:oh + 1], scale=a1[:, oh:oh + 1])
            for oh in range(2):
                g1f = [g1[b][ch][:].rearrange("p a b -> p (a b)") for ch in range(2)]
                c2f = c2[b][oh][:].rearrange("p a b -> p (a b)")
                for fh in range(2):
                    cp2 = c2ps.tile([128, 384], f32, tag="c2p", bufs=2, name=f"cp2_{b}_{oh}_{fh}")
                    for ch in range(2):
                        nc.tensor.matmul(cp2[:], w2[:, ch, oh * 128:(oh + 1) * 128],
                                         g1f[ch][:, fh * 384:(fh + 1) * 384],
                                         start=(ch == 0), stop=(ch == 1))
                    nc.vector.tensor_copy(c2f[:, fh * 384:(fh + 1) * 384], cp2[:])
                for ch in range(2):
                    nc.vector.bn_stats(st2[:, 2 * b + oh, ch, :],
                                       c2f[:, ch * 384:(ch + 1) * 384])
        mv2 = small.tile([128, 2, 2], f32)
        ar3b = small.tile([128, 4], f32)
        for oh in range(2):
            nc.vector.bn_aggr(mv2[:, oh, :], st2[:].rearrange("p (b o) c s -> p o b c s", o=2)[:, oh])
            nc.vector.tensor_scalar(ar3b[:, oh:oh + 1], mv2[:, oh, 0:1], 1536.0, None, ALU.mult)
            nc.vector.scalar_tensor_tensor(ar3b[:, 2 + oh:3 + oh], mv2[:, oh, 0:1],
                                           mv2[:, oh, 0:1], mv2[:, oh, 1:2], ALU.mult, ALU.add)
            nc.vector.tensor_scalar(ar3b[:, 2 + oh:3 + oh], ar3b[:, 2 + oh:3 + oh],
                                    1536.0, None, ALU.mult)
        ar3_i = dram.tile([128, 4], f32)
        ar3_o = dram.tile([128, 4], f32)
        nc.sync.dma_start(ar3_i[:], ar3b[:])
        nc.gpsimd.collective_compute("AllReduce", mybir.AluOpType.add,
                                     replica_groups=[list(range(NCORES))],
                                     ins=[ar3_i.opt()], outs=[ar3_o.opt()])

        # -------- b=1 exp groups that fill the AR3 latency --------
        for g in range(PREGROUPS):
            emit_exp_group(1, g, e2ps, csb)

        # -------- BN2 apply + max + y2 --------
        ar3r = small.tile([128, 4], f32)
        nc.sync.dma_start(ar3r[:], ar3_o[:])
        a2 = small.tile([128, 2], f32); b2 = small.tile([128, 2], f32)
        for oh in range(2):
            nc.vector.tensor_scalar(wk2[:, 0:1], ar3r[:, oh:oh + 1], 1.0 / CNT1, None, ALU.mult)
            nc.vector.tensor_scalar(wk2[:, 1:2], ar3r[:, 2 + oh:3 + oh], 1.0 / CNT1, None, ALU.mult)
            nc.vector.scalar_tensor_tensor(wk2[:, 2:3], wk2[:, 0:1], wk2[:, 0:1],
                                           wk2[:, 1:2], ALU.mult, ALU.subtract)
            nc.vector.tensor_scalar(wk2[:, 2:3], wk2[:, 2:3], -1.0, BNE, ALU.mult, ALU.add)  # var+eps
            nc.scalar.sqrt(wk2[:, 1:2], wk2[:, 2:3])
            nc.vector.reciprocal(wk2[:, 1:2], wk2[:, 1:2])
            nc.vector.tensor_tensor(a2[:, oh:oh + 1], wk2[:, 1:2], pp[:, G2_ + oh:G2_ + oh + 1], ALU.mult)
            nc.vector.tensor_scalar(wk2[:, 0:1], wk2[:, 0:1], -1.0, None, ALU.mult)
            nc.vector.scalar_tensor_tensor(b2[:, oh:oh + 1], a2[:, oh:oh + 1], wk2[:, 0:1],
                                           pp[:, BB2 + oh:BB2 + oh + 1], ALU.mult, ALU.add)
        y2 = [small.tile([128, 128], bf16, tag=f"y2_{b}_{kh}", name=f"y2_{b}_{kh}")
              for b in range(BLOC) for kh in range(2)]
        with tc.tile_pool(name="ph_d_ps", bufs=1, space="PSUM") as dps:
            for b in range(BLOC):
                gT = csb.tile([128, K], bf16, tag="gT", name=f"gT_{b}")
                for oh in range(2):
                    t2 = csb.tile([128, 6, 128], f32, tag="t2", name=f"t2_{b}_{oh}")
                    nc.scalar.activation(t2[:].rearrange("p a b -> p (a b)"),
                                         c2[b][oh][:].rearrange("p a b -> p (a b)"),
                                         AF.Identity, bias=b2[:, oh:oh + 1], scale=a2[:, oh:oh + 1])
                    gmx = csb.tile([128, 128], f32, tag="gmx", name=f"gmx_{b}_{oh}")
                    nc.vector.reduce_max(gmx[:], t2[:].rearrange("p a b -> p b a"),
                                         mybir.AxisListType.X)
                    grl = csb.tile([128, 128], bf16, tag="grl", name=f"grl_{b}_{oh}")
                    nc.scalar.activation(grl[:], gmx[:], AF.Relu)
                    gtp2 = dps.tile([128, 128], bf16, tag="gtp2", bufs=1, name=f"gtp2_{b}_{oh}")
                    nc.tensor.transpose(gtp2[:], grl[:], idb[:])
                    nc.vector.tensor_copy(gT[:, oh * 128:(oh + 1) * 128], gtp2[:])
                for kh in range(2):
                    yp = dps.tile([128, 128], f32, tag="yp", bufs=1, name=f"yp_{b}_{kh}")
                    nc.tensor.matmul(yp[:], gT[:, kh * 128:(kh + 1) * 128], scB[:])
                    nc.vector.tensor_copy(y2[b * 2 + kh][:], yp[:])

    # ---------------- final: out = scA@x + y2@S2 + sc_b ----------------
    with (
        tc.tile_pool(name="ph_e_ps", bufs=1, space="PSUM") as fps_p,
        tc.tile_pool(name="ph_e_sb", bufs=2) as esb,
    ):
        for t in range(NT):
            emit_final(0, t, fps_p)
        for g in range(NT):
            if g >= PREGROUPS:
                emit_exp_group(1, g, fps_p, esb)
            emit_transpose_group(1, g)
        for t in range(NT):
            emit_final(1, t, fps_p)
    es.close()


def _build(reuse_h: bool, dub_zero: bool, iters: int = 1):
    key = (reuse_h, dub_zero, iters)
    if key in _cache:
        return _cache[key]
    nc = bacc.Bacc("TRN2", target_bir_lowering=False, debug=False,
                   num_devices=NCORES)
    with tile.TileContext(nc) as tc:
        _emit(nc, tc, (reuse_h, dub_zero), iters=iters)
    nc.compile()
    _cache[key] = nc
    return nc


def prepare(inputs):
    """Host-side prep: returns (reuse_h, dub_zero, in_maps)."""
    x = np.asarray(inputs['data'])[..., 0]          # [B, C, N] f32
    dp_gamma = np.asarray(inputs['dp_gamma']); dp_beta = np.asarray(inputs['dp_beta'])
    du_gamma = np.asarray(inputs['du_gamma']); du_beta = np.asarray(inputs['du_beta'])
    dp_w = np.asarray(inputs['dp_w']); du_w = np.asarray(inputs['du_w'])
    du_b = np.asarray(inputs['du_b'])
    dg_w1 = np.asarray(inputs['dg_w1']); dg_w2 = np.asarray(inputs['dg_w2'])
    dg_g1 = np.asarray(inputs['dg_g1']); dg_bb1 = np.asarray(inputs['dg_bb1'])
    dg_g2 = np.asarray(inputs['dg_g2']); dg_bb2 = np.asarray(inputs['dg_bb2'])
    sc_w = np.asarray(inputs['sc_w']); sc_b = np.asarray(inputs['sc_b'])

    reuse_h = bool(np.array_equal(dp_gamma, du_gamma) and np.array_equal(dp_beta, du_beta))
    dub_zero = bool(np.all(du_b == 0))

    bf = ml_dtypes.bfloat16
    x_bf = x.astype(bf)
    pp = np.zeros((128, 13), np.float32)
    pp[:, 0] = dp_gamma; pp[:, 1] = dp_beta; pp[:, 2] = du_gamma; pp[:, 3] = du_beta
    pp[:, 4] = sc_b
    for hh in range(2):
        sl = slice(hh * 128, (hh + 1) * 128)
        pp[:, 5 + hh] = dg_g1[sl]; pp[:, 7 + hh] = dg_bb1[sl]
        pp[:, 9 + hh] = dg_g2[sl]; pp[:, 11 + hh] = dg_bb2[sl]

    const_map = {
        "dpw_t": np.ascontiguousarray(dp_w.T).astype(bf),
        "duw_t": np.ascontiguousarray(du_w.T).astype(bf),
        "scA_t": np.ascontiguousarray(sc_w[:, :128].T).astype(bf),
        "scB_t": np.ascontiguousarray(sc_w[:, 128:].T).astype(bf),
        "w1s_t": np.ascontiguousarray((dg_w1[:, :256] + dg_w1[:, 256:]).T).astype(bf),
        "w1b_t": np.ascontiguousarray(dg_w1[:, 256:].T).astype(bf),
        "w2_t": np.ascontiguousarray(dg_w2.T).astype(bf),
        "identf": np.eye(128, dtype=np.float32),
        "identb": np.eye(128).astype(bf),
        "pp": pp,
        "dub_bc": np.broadcast_to(du_b[None, :], (128, 256)).astype(np.float32).copy(),
    }
    in_maps = []
    for c in range(NCORES):
        m = dict(const_map)
        m["xb"] = np.ascontiguousarray(x_bf[c * BLOC:(c + 1) * BLOC])
        in_maps.append(m)
    return reuse_h, dub_zero, in_maps


def kernel(**inputs) -> np.ndarray:
    reuse_h, dub_zero, in_maps = prepare(inputs)
    nc = _build(reuse_h, dub_zero, ITERS)
    global LAST_RESULT
    res = run_bass_kernel_spmd(nc, in_maps, core_ids=list(range(NCORES)),
                               trace=TRACE, **(TRACE_KW or {}))
    LAST_RESULT = res
    out = np.concatenate([res.results[c]["out"] for c in range(NCORES)], axis=0)
    return out[..., None].astype(np.float32)


if __name__ == "__main__":
    import reference
    ins = {k: np.asarray(v) for k, v in reference.setup_inputs().items()}
    got = kernel(**ins)
    print("out shape", got.shape, got.dtype)
